# revision 1
# baseline (speedup 1.0000x reference)
"""Trainium2 Bass kernel for nn_MessageLayer (GNN message passing), 8 NeuronCores.

Reference computation:
    edge_mat = (edge_features @ W + b).reshape(E, 64, 16)
    messages = einsum('emh,eh->em', edge_mat, hidden[edge_sources])
    out      = segment_sum(messages, edge_targets, num_segments=10000)

Algebraic restructure (cuts FLOPs 32x): since aggregation is linear,
    out[n, m] = sum_{f,h} W[f, m*16+h] * C[n, f, h],
    C[n, f, h] = sum_{e: tgt(e)=n} ef[e, f] * hidden[src(e), h]
C is built with one tiny PE matmul per target node (lhsT = that node's edge
features [deg, 32], rhs = gathered source hidden [deg, 16]) — segment
boundaries are known when the kernel is built, so they are baked into the
unrolled instruction stream.  Then out = C @ Wr as 32 accumulating matmuls
against a block-diagonal-packed W.

Sharding: node-ownership (scatter-reduce by target): core c owns nodes
[1250c, 1250c+1250) and receives exactly the edges targeting them, so no
collective is needed; each core writes its own output rows.  The single SPMD
program is made core-uniform by sorting each core's segments by length and
padding each position to K_j = max over cores of the j-th longest segment.

Hardware constraint discovered empirically: matmuls whose stationary tiles
sit on disjoint row-groups execute CONCURRENTLY, and two concurrent matmuls
draining into the same PSUM bank wedge the device (NRT_EXEC_UNIT_
UNRECOVERABLE).  C-bank assignment is therefore keyed by starting row-group:
matmuls sharing a row-group serialize on the PE and may share a bank;
different row-groups always land in different banks.
"""
import numpy as np
from contextlib import ExitStack

N_NODES = 10000
N_EDGES = 320000
HID = 16
MSG = 64
EFD = 32
NCORES = 8
NPC = N_NODES // NCORES          # 1250 nodes owned per core
NCHUNK = 5                       # input DMA chunks
CPBUFS = 6                       # PSUM tiles for C banks (+2 for the W stage)

_CACHE = {}


def _build_layout(edge_targets):
    """Per-core segment lists + the shared (SPMD-uniform) layout."""
    segs_per_core = []
    for c in range(NCORES):
        lo = c * NPC
        mask = (edge_targets >= lo) & (edge_targets < lo + NPC)
        eids = np.nonzero(mask)[0]
        tgt = edge_targets[eids]
        order = np.argsort(tgt, kind="stable")
        eids = eids[order]
        tgt = tgt[order]
        segs = []
        uniq, starts = np.unique(tgt, return_index=True)
        bounds = list(starts) + [len(tgt)]
        for i, n in enumerate(uniq):
            s, e = bounds[i], bounds[i + 1]
            while e - s > 128:          # split over-long segments; host re-adds
                segs.append((int(n), eids[s:s + 128]))
                s += 128
            segs.append((int(n), eids[s:e]))
        segs.sort(key=lambda t: -len(t[1]))
        segs_per_core.append(segs)

    nseg = max(len(s) for s in segs_per_core)
    NPOS = ((nseg + 127) // 128) * 128
    K = np.ones(NPOS, dtype=np.int64)
    for segs in segs_per_core:
        for j, (_, e) in enumerate(segs):
            K[j] = max(K[j], len(e))

    # pack positions into 128-row tiles at 32-aligned row bases
    t_j = np.zeros(NPOS, dtype=np.int64)
    r_j = np.zeros(NPOS, dtype=np.int64)
    tile, row = 0, 0
    for j in range(NPOS):
        s = (int(K[j]) + 31) // 32
        if row + 32 * s > 128:
            tile += 1
            row = 0
        t_j[j], r_j[j] = tile, row
        row += 32 * s
        if row == 128:
            tile += 1
            row = 0
    T = tile + (1 if row > 0 else 0)

    # PSUM-bank assignment keyed by starting row-group (see module docstring):
    # four bank streams, each filling (q, w) slots; a full bank is copied out
    # and the stream opens a fresh one.
    pos_bank = np.zeros(NPOS, dtype=np.int64)
    pos_q = np.zeros(NPOS, dtype=np.int64)
    pos_w = np.zeros(NPOS, dtype=np.int64)
    stream_bank = [-1, -1, -1, -1]
    stream_cnt = [0, 0, 0, 0]
    next_bank = 0
    for j in range(NPOS):
        g = int(r_j[j]) // 32
        if stream_bank[g] < 0:
            stream_bank[g] = next_bank
            next_bank += 1
            stream_cnt[g] = 0
        cnt = stream_cnt[g]
        pos_bank[j] = stream_bank[g]
        pos_q[j] = cnt // 32
        pos_w[j] = cnt % 32
        stream_cnt[g] = cnt + 1
        if stream_cnt[g] == 128:
            stream_bank[g] = -1
    NB = next_bank
    return segs_per_core, NPOS, K, t_j, r_j, T, pos_bank, pos_q, pos_w, NB


def _pack_core(segs, NPOS, K, t_j, r_j, T, wbd, edge_features, edge_sources,
               hidden):
    # combined row data: 48 cols per row = 32 edge features + 16 source hidden
    D = np.zeros((T * 128, EFD + HID), dtype=np.float32)
    for j in range(min(len(segs), NPOS)):
        _, eids = segs[j]
        base = t_j[j] * 128 + r_j[j]
        D[base:base + len(eids), :EFD] = edge_features[eids]
        D[base:base + len(eids), EFD:] = hidden[edge_sources[eids]]
    # DRAM layout: [128 partitions, T*48 + 4096] so each partition is one
    # contiguous DMA span; SBUF tile t sits at free offset t*48; the
    # block-diag W rides in the tail so it shares the last chunk's DMA.
    d = D.reshape(T, 128, EFD + HID).swapaxes(0, 1).reshape(128, T * (EFD + HID))
    return np.ascontiguousarray(np.concatenate([d, wbd], axis=1))


def _build_wbd(W):
    # Wbd[p=2h+half] [(q,f)=128, (q',mh)=128] = delta_qq' W[f, (mh+32*half)*16+h]
    wbd = np.zeros((32, 128, 128), dtype=np.float32)
    Wr = W.reshape(EFD, MSG, HID)                      # [f, m, h]
    for h in range(HID):
        for half in range(2):
            p = 2 * h + half
            blk = Wr[:, 32 * half:32 * half + 32, h]   # [f=32, mh=32]
            for q in range(4):
                wbd[p, 32 * q:32 * q + 32, 32 * q:32 * q + 32] = blk
    # DRAM layout [128, 32*128]: phase p at free offset 128p
    return np.ascontiguousarray(wbd.transpose(1, 0, 2).reshape(128, 32 * 128))


def _chunk_bounds(T):
    return [round(k * T / NCHUNK) for k in range(NCHUNK)] + [T]


def _build_program(NPOS, K, t_j, r_j, T, pos_bank, pos_q, pos_w, NB):
    import concourse.tile as tile
    from concourse import bacc, mybir

    RW = EFD + HID                   # 48 row cols (ef | nh)
    f32 = mybir.dt.float32
    bounds = _chunk_bounds(T)

    nc = bacc.Bacc("TRN2", target_bir_lowering=False, debug=False,
                   num_devices=NCORES)
    data_dram = nc.dram_tensor("data", [128, T * RW + 32 * 128], f32,
                               kind="ExternalInput").ap()
    out_dram = nc.dram_tensor("out", [128, 2 * NB * 32], f32,
                              kind="ExternalOutput").ap()

    with tile.TileContext(nc) as tc, ExitStack() as ctx:
        big = ctx.enter_context(tc.tile_pool(name="big", bufs=1))
        cpool = ctx.enter_context(tc.tile_pool(name="cps", bufs=CPBUFS,
                                               space="PSUM"))
        opool = ctx.enter_context(tc.tile_pool(name="ops", bufs=1, space="PSUM"))

        ch_sb = []
        for k in range(NCHUNK):
            lo, hi = bounds[k] * RW, bounds[k + 1] * RW
            if k == NCHUNK - 1:
                hi += 32 * 128       # wbd tail rides with the last chunk
            t = big.tile([128, hi - lo], f32, tag=f"ch{k}", name=f"ch{k}")
            nc.sync.dma_start(t[:], data_dram[:, lo:hi])
            ch_sb.append(t)
        wbd_sb = ch_sb[-1][:, (bounds[NCHUNK] - bounds[NCHUNK - 1]) * RW:]

        c_all = big.tile([128, NB * 512], f32, tag="call")

        # final bank of each row-group stream may be partially filled: zero
        # its c_all region so the prefix-copy below leaves no stale data
        gcount = {}
        for j in range(NPOS):
            b = int(pos_bank[j])
            gcount[b] = gcount.get(b, 0) + 1
        for b, cnt in gcount.items():
            if cnt < 128:
                nc.vector.memset(c_all[:, 512 * b:512 * (b + 1)], 0.0)

        chunk_of = np.searchsorted(np.array(bounds[1:]), t_j, side="right")
        stream_tile = [None, None, None, None]
        stream_n = [0, 0, 0, 0]
        for j in range(NPOS):
            g = int(r_j[j]) // 32
            if stream_tile[g] is None:
                stream_tile[g] = cpool.tile([128, 512], f32, tag="cps",
                                            name=f"cps_b{int(pos_bank[j])}")
                stream_n[g] = 0
            t, r, kk = int(t_j[j]), int(r_j[j]), int(K[j])
            ch = int(chunk_of[j])
            base = (t - bounds[ch]) * RW
            q, w = int(pos_q[j]), int(pos_w[j])
            lhsT = ch_sb[ch][r:r + kk, base:base + EFD]
            rhs = ch_sb[ch][r:r + kk, base + EFD:base + RW]
            out = stream_tile[g][32 * q:32 * q + 32, 16 * w:16 * w + 16]
            nc.tensor.matmul(out, lhsT, rhs, start=True, stop=True,
                             tile_position=(r, 32 * q))
            stream_n[g] += 1
            if stream_n[g] == 128:
                b = int(pos_bank[j])
                nc.vector.tensor_copy(c_all[:, 512 * b:512 * b + 512],
                                      stream_tile[g][:])
                stream_tile[g] = None
        for g in range(4):           # flush partial final banks
            if stream_tile[g] is not None:
                b = [int(pos_bank[j]) for j in range(NPOS)
                     if int(r_j[j]) // 32 == g][-1]
                nc.vector.tensor_copy(c_all[:, 512 * b:512 * b + 512],
                                      stream_tile[g][:])

        out_sb = big.tile([128, 2 * NB * 32], f32, tag="outsb")
        for half in range(2):
            po = opool.tile([128, NB * 32], f32, tag=f"po{half}",
                            name=f"po{half}")
            for h in range(HID):
                p = 2 * h + half
                nc.tensor.matmul(
                    po[:], wbd_sb[:, 128 * p:128 * p + 128], c_all[:, h::16],
                    start=(h == 0), stop=(h == HID - 1))
            nc.vector.tensor_copy(
                out_sb[:, NB * 32 * half:NB * 32 * (half + 1)], po[:])
        nc.sync.dma_start(out_dram[:], out_sb[:])
    nc.compile()
    return nc


def _assemble(outs, segs_per_core, NPOS, pos_bank, pos_q, pos_w, NB):
    WND = NB * 32
    out = np.zeros((N_NODES, MSG), dtype=np.float32)
    for c in range(NCORES):
        out_sb = outs[c]
        pos_rows = np.empty((NPOS, MSG), dtype=np.float32)
        for half in range(2):
            pos_rows[:, 32 * half:32 * half + 32] = \
                out_sb[32 * pos_q[:, None] + np.arange(32)[None, :],
                       (WND * half + 32 * pos_bank + pos_w)[:, None]]
        segs = segs_per_core[c]
        for j in range(min(len(segs), NPOS)):
            n, _ = segs[j]
            out[n] += pos_rows[j]
    return out


def kernel(node_features, edge_features, edge_sources, edge_targets,
           hidden, initial, W, b):
    from concourse.bass_utils import run_bass_kernel_spmd

    edge_targets = np.asarray(edge_targets)
    edge_sources = np.asarray(edge_sources)
    edge_features = np.asarray(edge_features, dtype=np.float32)
    hidden = np.asarray(hidden, dtype=np.float32)
    W = np.asarray(W, dtype=np.float32)
    b = np.asarray(b, dtype=np.float32)

    key = edge_targets.tobytes()
    if key in _CACHE:
        layout, nc = _CACHE[key]
    else:
        layout = _build_layout(edge_targets)
        segs_per_core, NPOS, K, t_j, r_j, T, pos_bank, pos_q, pos_w, NB = layout
        assert K.max() <= 128
        nc = _build_program(NPOS, K, t_j, r_j, T, pos_bank, pos_q, pos_w, NB)
        _CACHE[key] = (layout, nc)
    segs_per_core, NPOS, K, t_j, r_j, T, pos_bank, pos_q, pos_w, NB = layout

    wbd = _build_wbd(W)
    in_maps = []
    for c in range(NCORES):
        data = _pack_core(segs_per_core[c], NPOS, K, t_j, r_j, T, wbd,
                          edge_features, edge_sources, hidden)
        in_maps.append({"data": data})

    res = run_bass_kernel_spmd(nc, in_maps, list(range(NCORES)))
    outs = [res.results[c]["out"] for c in range(NCORES)]
    out = _assemble(outs, segs_per_core, NPOS, pos_bank, pos_q, pos_w, NB)

    if np.any(b):
        # bias term: out[n] += (sum_{e->n} hidden[src e]) @ Br,
        # Br[h, m] = b[m*16+h].  (b is all-zero for this problem.)
        Br = b.reshape(MSG, HID).T.astype(np.float32)
        acc = np.zeros((N_NODES, HID), dtype=np.float32)
        np.add.at(acc, edge_targets, hidden[edge_sources])
        out += acc @ Br
    return out



# revision 4
# speedup vs baseline: 1.3140x; 1.3140x over previous
"""Trainium2 Bass kernel for nn_MessageLayer (GNN message passing), 8 NeuronCores.

Reference computation:
    edge_mat = (edge_features @ W + b).reshape(E, 64, 16)
    messages = einsum('emh,eh->em', edge_mat, hidden[edge_sources])
    out      = segment_sum(messages, edge_targets, num_segments=10000)

Algebraic restructure (cuts FLOPs 32x): since aggregation is linear,
    out[n, m] = sum_{f,h} W[f, m*16+h] * C[n, f, h],
    C[n, f, h] = sum_{e: tgt(e)=n} ef[e, f] * hidden[src(e), h]

v2 structure (vs the v1 per-segment mini-matmul kernel, which was
tensor-engine-instruction-bound: 2624 LDWEIGHTS+MATMUL pairs of ~370 ns
drain-latency-serialized work per core):

Each segment is split into chunks of <=32 edges ("positions"), sorted by
length.  One full-array matmul handles 16 positions at once:
  - K = 128 rows = 4 row-slots of 32 (slot j holds a "quad" = 4 positions)
  - stationary [128, 128]: row r of slot j carries the edge features of the
    4 positions (classes g=0..3) of quad 4t+j at column groups 32g..32g+32
    -- fully dense.
  - moving [128, 256]: slot j's rows carry the 4 source-hidden vectors at
    cols 64j + 16g + h; all other moving entries are ZERO (slot separation),
    provided by an SBUF memset + per-slot compact DMA.
  - PSUM out [128, 256]: block (32g+f, 64j+16g+h) = C[pos(t,j,g), f, h];
    off-diagonal (g,g') blocks are garbage and dropped by the drain.
Drains: per PSUM bank (2 matmuls) and class g, one strided copy
  [32, 8 stripes of 16] -> dense bf16 c_all[32g+f, (t,m,j)*16 + h],
  rotated across vector/scalar/gpsimd engines.
W-stage: out = C @ W via 32 accumulating full-array matmuls against a
block-diagonal-packed W (bf16), exactly as v1 but denser.

Sharding: node-ownership (scatter-reduce by target): core c owns nodes
[1250c, 1250c+1250) and receives exactly the edges targeting them, so no
collective is needed; host assembles per-position rows into final output.
All tensors bf16 on the wire/SBUF (f32 PSUM accumulate), which more than
halves DMA and doubles PE weight-load rate; rel-err ~1e-3 vs the 2e-2 gate.
"""
import numpy as np
from contextlib import ExitStack

N_NODES = 10000
N_EDGES = 320000
HID = 16
MSG = 64
EFD = 32
NCORES = 8
NPC = N_NODES // NCORES          # 1250 nodes owned per core
SPLIT = 32                       # max edges per position (row-slot height)
NCH = 4                          # DMA chunks for compute/DMA overlap
CPBUFS = 6                       # PSUM tiles for C banks (+2 for the W stage)

_CACHE = {}


def _bf16():
    import ml_dtypes
    return ml_dtypes.bfloat16


def _build_layout(edge_targets):
    """Per-core position lists (node, edge-ids, len<=32, sorted desc) plus
    the shared SPMD-uniform grid size: T matmuls of 16 positions each."""
    segs_per_core = []
    for c in range(NCORES):
        lo = c * NPC
        mask = (edge_targets >= lo) & (edge_targets < lo + NPC)
        eids = np.nonzero(mask)[0]
        tgt = edge_targets[eids]
        order = np.argsort(tgt, kind="stable")
        eids = eids[order]
        tgt = tgt[order]
        segs = []
        uniq, starts = np.unique(tgt, return_index=True)
        bounds = list(starts) + [len(tgt)]
        for i, n in enumerate(uniq):
            s, e = bounds[i], bounds[i + 1]
            while e - s > SPLIT:
                segs.append((int(n), eids[s:s + SPLIT]))
                s += SPLIT
            segs.append((int(n), eids[s:e]))
        segs.sort(key=lambda t: -len(t[1]))
        segs_per_core.append(segs)

    nseg = max(len(s) for s in segs_per_core)
    NPOS = ((nseg + 31) // 32) * 32      # multiple of 32 -> T even
    T = NPOS // 16                       # one matmul per 16 positions
    U = NPOS // 4                        # quads; po free width per half
    assert U <= 512, f"U={U} exceeds one PSUM bank"
    return segs_per_core, NPOS, T, U


def _build_wbd(W):
    # Wbd[p=2h+half] [(g,f)=128, (g,mh)=128] = delta_gg' W[f, (mh+32*half)*16+h]
    wbd = np.zeros((32, 128, 128), dtype=np.float32)
    Wr = W.reshape(EFD, MSG, HID)                      # [f, m, h]
    for h in range(HID):
        for half in range(2):
            p = 2 * h + half
            blk = Wr[:, 32 * half:32 * half + 32, h]   # [f=32, mh=32]
            for g in range(4):
                wbd[p, 32 * g:32 * g + 32, 32 * g:32 * g + 32] = blk
    # DRAM layout [128, 32*128]: phase p at free offset 128p
    return np.ascontiguousarray(wbd.transpose(1, 0, 2).reshape(128, 32 * 128))


def _pack_core(segs, NPOS, T, wbd, edge_features, edge_sources, hidden):
    """DRAM image per core: [128, T*128 (stationary) | T*64 (moving compact)
    | 32*128 (wbd)], bf16.  Position p=(t=p//16, j=(p//4)%4, g=p%4):
      stationary[32j+r, t*128+32g+f] = ef[E_p[r], f]
      moving-compact[32j+r, t*64+16g+h] = hidden[src(E_p[r]), h]
    (moving band j lives at partitions 32j..32j+32 of the same col range)."""
    St = np.zeros((128, T * 128), dtype=np.float32)
    Mv = np.zeros((128, T * 64), dtype=np.float32)
    for p in range(min(len(segs), NPOS)):
        _, eids = segs[p]
        k = len(eids)
        t, j, g = p // 16, (p // 4) % 4, p % 4
        r0 = 32 * j
        St[r0:r0 + k, t * 128 + 32 * g:t * 128 + 32 * g + EFD] = \
            edge_features[eids]
        Mv[r0:r0 + k, t * 64 + 16 * g:t * 64 + 16 * g + HID] = \
            hidden[edge_sources[eids]]
    D = np.concatenate([St, Mv, wbd], axis=1)
    return np.ascontiguousarray(D.astype(_bf16()))


def _chunk_bounds(T):
    return [round(k * T / NCH) for k in range(NCH)] + [T]


def _build_program(T, U):
    import concourse.tile as tile
    from concourse import bacc, mybir

    f32 = mybir.dt.float32
    bf16 = mybir.dt.bfloat16
    ST_W = T * 128
    MV_W = T * 64
    bounds = _chunk_bounds(T)

    nc = bacc.Bacc("TRN2", target_bir_lowering=False, debug=False,
                   num_devices=NCORES)
    data_dram = nc.dram_tensor("data", [128, ST_W + MV_W + 32 * 128], bf16,
                               kind="ExternalInput").ap()
    out_dram = nc.dram_tensor("out", [128, 2 * U], f32,
                              kind="ExternalOutput").ap()

    with tile.TileContext(nc) as tc, ExitStack() as ctx:
        big = ctx.enter_context(tc.tile_pool(name="big", bufs=1))
        cpool = ctx.enter_context(tc.tile_pool(name="cps", bufs=CPBUFS,
                                               space="PSUM"))
        opool = ctx.enter_context(tc.tile_pool(name="ops", bufs=1,
                                               space="PSUM"))

        st_sb = big.tile([128, ST_W], bf16, tag="st")
        mv_sb = big.tile([128, T * 256], bf16, tag="mv")
        wbd_sb = big.tile([128, 32 * 128], bf16, tag="wbd")
        c_all = big.tile([128, U * 16], bf16, tag="call")
        out_sb = big.tile([128, 2 * U], f32, tag="outsb")

        # per chunk: zero the moving region (slot-separation zeros), then
        # DMA the stationary block + the 4 compact moving bands into it
        for kc in range(NCH):
            b0, b1 = bounds[kc], bounds[kc + 1]
            mid = (b0 + b1) // 2
            nc.vector.memset(mv_sb[:, b0 * 256:mid * 256], 0.0)
            nc.gpsimd.memset(mv_sb[:, mid * 256:b1 * 256], 0.0)
            nc.sync.dma_start(st_sb[:, b0 * 128:b1 * 128],
                              data_dram[:, b0 * 128:b1 * 128])
            for j in range(4):
                dst = mv_sb[32 * j:32 * j + 32, b0 * 256:b1 * 256] \
                    .rearrange("p (t w) -> p t w", w=256)[:, :, 64 * j:64 * j + 64]
                src = data_dram[32 * j:32 * j + 32,
                                ST_W + b0 * 64:ST_W + b1 * 64]
                nc.sync.dma_start(dst, src)
        nc.sync.dma_start(wbd_sb[:], data_dram[:, ST_W + MV_W:])

        # C stage: one full-array matmul per 16 positions; drain each PSUM
        # bank (2 matmuls) with 4 strided class-copies rotated over engines
        # gpsimd cannot read PSUM; split the 4 class-drains DVE/ACT 2+2
        drain_engines = [nc.vector.tensor_copy, nc.scalar.copy,
                         nc.vector.tensor_copy, nc.scalar.copy]
        ps = None
        for t in range(T):
            if t % 2 == 0:
                ps = cpool.tile([128, 512], f32, tag="cps")
            nc.tensor.matmul(ps[:, 256 * (t % 2):256 * (t % 2) + 256],
                             st_sb[:, t * 128:(t + 1) * 128],
                             mv_sb[:, t * 256:(t + 1) * 256],
                             start=True, stop=True)
            if t % 2 == 1:
                b = t // 2
                src3 = ps[:].rearrange("p (s w) -> p s w", w=16)
                for g in range(4):
                    dst = c_all[32 * g:32 * g + 32, b * 128:(b + 1) * 128] \
                        .rearrange("p (s w) -> p s w", w=16)
                    drain_engines[g](dst, src3[32 * g:32 * g + 32, g::4, :])

        # W stage: po[32g+mh, u] = sum_{f,h} W[f,(mh+32half)*16+h] C[u,g,f,h]
        for half in range(2):
            po = opool.tile([128, U], f32, tag=f"po{half}", name=f"po{half}")
            for h in range(HID):
                p = 2 * h + half
                nc.tensor.matmul(po[:], wbd_sb[:, 128 * p:128 * p + 128],
                                 c_all[:, h::16],
                                 start=(h == 0), stop=(h == HID - 1))
            nc.vector.tensor_copy(out_sb[:, U * half:U * (half + 1)], po[:])
        nc.sync.dma_start(out_dram[:], out_sb[:])
    nc.compile()
    return nc


def _assemble(outs, segs_per_core, NPOS, U):
    out = np.zeros((N_NODES, MSG), dtype=np.float32)
    mw = np.arange(32)[None, :]
    for c in range(NCORES):
        segs = segs_per_core[c]
        P = min(len(segs), NPOS)
        if P == 0:
            continue
        po_sb = outs[c].astype(np.float32)           # [128, 2U]
        p = np.arange(P)
        u, g = p // 4, p % 4
        part = 32 * g[:, None] + mw                  # [P, 32]
        pos_rows = np.empty((P, MSG), dtype=np.float32)
        pos_rows[:, :32] = po_sb[part, u[:, None]]
        pos_rows[:, 32:] = po_sb[part, U + u[:, None]]
        nodes = np.fromiter((segs[i][0] for i in range(P)), dtype=np.int64,
                            count=P)
        np.add.at(out, nodes, pos_rows)
    return out


def kernel(node_features, edge_features, edge_sources, edge_targets,
           hidden, initial, W, b):
    from concourse.bass_utils import run_bass_kernel_spmd

    edge_targets = np.asarray(edge_targets)
    edge_sources = np.asarray(edge_sources)
    edge_features = np.asarray(edge_features, dtype=np.float32)
    hidden = np.asarray(hidden, dtype=np.float32)
    W = np.asarray(W, dtype=np.float32)
    b = np.asarray(b, dtype=np.float32)

    key = edge_targets.tobytes()
    if key in _CACHE:
        layout, nc = _CACHE[key]
    else:
        layout = _build_layout(edge_targets)
        segs_per_core, NPOS, T, U = layout
        nc = _build_program(T, U)
        _CACHE[key] = (layout, nc)
    segs_per_core, NPOS, T, U = layout

    wbd = _build_wbd(W)
    in_maps = []
    for c in range(NCORES):
        data = _pack_core(segs_per_core[c], NPOS, T, wbd,
                          edge_features, edge_sources, hidden)
        in_maps.append({"data": data})

    res = run_bass_kernel_spmd(nc, in_maps, list(range(NCORES)))
    outs = [res.results[c]["out"] for c in range(NCORES)]
    out = _assemble(outs, segs_per_core, NPOS, U)

    if np.any(b):
        # bias term: out[n] += (sum_{e->n} hidden[src e]) @ Br,
        # Br[h, m] = b[m*16+h].  (b is all-zero for this problem.)
        Br = b.reshape(MSG, HID).T.astype(np.float32)
        acc = np.zeros((N_NODES, HID), dtype=np.float32)
        np.add.at(acc, edge_targets, hidden[edge_sources])
        out += acc @ Br
    return out


# revision 7
# speedup vs baseline: 1.6461x; 1.2527x over previous
"""Trainium2 Bass kernel for nn_MessageLayer (GNN message passing), 8 NeuronCores.

Reference computation:
    edge_mat = (edge_features @ W + b).reshape(E, 64, 16)
    messages = einsum('emh,eh->em', edge_mat, hidden[edge_sources])
    out      = segment_sum(messages, edge_targets, num_segments=10000)

Algebraic restructure (cuts FLOPs 32x): since aggregation is linear,
    out[n, m] = sum_{f,h} W[f, m*16+h] * C[n, f, h],
    C[n, f, h] = sum_{e: tgt(e)=n} ef[e, f] * hidden[src(e), h]

Structure (v2.1):
Each segment is split into chunks of <=32 edges ("positions"), sorted by
length.  One full-array matmul handles 16 positions at once:
  - K = 128 rows = 4 row-slots of 32 (slot j holds a "quad" = 4 positions)
  - stationary [128, 128]: row r of slot j carries the edge features of the
    4 positions (classes g=0..3) of quad 4t+j at column groups 32g..32g+32
    -- fully dense.
  - moving [128, 256]: slot j's rows carry the 4 source-hidden vectors at
    cols 64j + 16g + h; all other moving entries are ZERO (slot separation),
    provided by a gpsimd memset + per-slot compact DMA.
  - PSUM out [128, 256]: block (32g+f, 64j+16g+h) = C[pos(t,j,g), f, h];
    (g,g') off-diagonal blocks are garbage that simply stays in place.
Drains are single full-width [128, 512] f32->bf16 copies per PSUM bank
(alternating DVE/ACT) into a *spread* c_spread; the W-stage then runs as
4 concurrent 32-row-strip matmul chains (one per class g), each reading
its valid columns directly via the stride-64 pattern
    c_spread[32g:32g+32, (16g+h)::64]   ->  [32, U] per phase h,
against a 4x-replicated W stationary [32f@32g, 64m] (both m-halves at
once, 16 accumulating phases into po_g [64, U] PSUM).

Sharding: node-ownership (scatter-reduce by target): core c owns nodes
[1250c, 1250c+1250) and receives exactly the edges targeting them, so no
collective is needed; host assembles per-position rows into final output.
All tensors bf16 on the wire/SBUF (f32 PSUM accumulate): rel-err ~3.5e-3
vs the 2e-2 gate.
"""
import numpy as np
from contextlib import ExitStack

N_NODES = 10000
N_EDGES = 320000
HID = 16
MSG = 64
EFD = 32
NCORES = 8
NPC = N_NODES // NCORES          # 1250 nodes owned per core
SPLIT = 32                       # max edges per position (row-slot height)
CPBUFS = 4                       # PSUM tiles for C banks (4 + 4 po = 8)

_CACHE = {}


def _bf16():
    import ml_dtypes
    return ml_dtypes.bfloat16


def _build_layout(edge_targets):
    """Per-core position lists (node, edge-ids, len<=32, sorted desc) plus
    the shared SPMD-uniform grid size: T matmuls of 16 positions each."""
    segs_per_core = []
    for c in range(NCORES):
        lo = c * NPC
        mask = (edge_targets >= lo) & (edge_targets < lo + NPC)
        eids = np.nonzero(mask)[0]
        tgt = edge_targets[eids]
        order = np.argsort(tgt, kind="stable")
        eids = eids[order]
        tgt = tgt[order]
        segs = []
        uniq, starts = np.unique(tgt, return_index=True)
        bounds = list(starts) + [len(tgt)]
        for i, n in enumerate(uniq):
            s, e = bounds[i], bounds[i + 1]
            while e - s > SPLIT:
                segs.append((int(n), eids[s:s + SPLIT]))
                s += SPLIT
            segs.append((int(n), eids[s:e]))
        segs.sort(key=lambda t: -len(t[1]))
        segs_per_core.append(segs)

    nseg = max(len(s) for s in segs_per_core)
    NPOS = ((nseg + 31) // 32) * 32      # multiple of 32 -> T even
    T = NPOS // 16                       # one matmul per 16 positions
    U = NPOS // 4                        # quads; po free width
    assert U <= 512, f"U={U} exceeds one PSUM bank"
    return segs_per_core, NPOS, T, U


def _build_w2(W):
    # w2[32g+f, 64h+m] = W[f, m*16+h], replicated across the 4 class groups
    Wr = W.reshape(EFD, MSG, HID).transpose(0, 2, 1)   # [f, h, m]
    blk = np.ascontiguousarray(Wr.reshape(EFD, HID * MSG))
    return np.tile(blk, (4, 1)).astype(np.float32)     # [128, 1024]


def _pack_core(segs, NPOS, T, w2, edge_features, edge_sources, hidden):
    """DRAM image per core: [128, T*128 (stationary) | T*64 (moving compact)
    | 1024 (w2)], bf16.  Position p=(t=p//16, j=(p//4)%4, g=p%4):
      stationary[32j+r, t*128+32g+f] = ef[E_p[r], f]
      moving-compact[32j+r, t*64+16g+h] = hidden[src(E_p[r]), h]
    (moving band j lives at partitions 32j..32j+32 of the same col range)."""
    St = np.zeros((128, T * 128), dtype=np.float32)
    Mv = np.zeros((128, T * 64), dtype=np.float32)
    for p in range(min(len(segs), NPOS)):
        _, eids = segs[p]
        k = len(eids)
        t, j, g = p // 16, (p // 4) % 4, p % 4
        r0 = 32 * j
        St[r0:r0 + k, t * 128 + 32 * g:t * 128 + 32 * g + EFD] = \
            edge_features[eids]
        Mv[r0:r0 + k, t * 64 + 16 * g:t * 64 + 16 * g + HID] = \
            hidden[edge_sources[eids]]
    D = np.concatenate([St, Mv, w2], axis=1)
    return np.ascontiguousarray(D.astype(_bf16()))


def _chunk_bounds(T):
    # small first chunk so the PE starts early
    return [0, max(2, T // 8) & ~1, (T // 3) & ~1, (2 * T // 3) & ~1, T]


def _build_program(T, U):
    import concourse.tile as tile
    from concourse import bacc, mybir

    f32 = mybir.dt.float32
    bf16 = mybir.dt.bfloat16
    ST_W = T * 128
    MV_W = T * 64
    B = T // 2                           # PSUM banks over the C stage
    bounds = _chunk_bounds(T)
    NCH = len(bounds) - 1

    nc = bacc.Bacc("TRN2", target_bir_lowering=False, debug=False,
                   num_devices=NCORES)
    data_dram = nc.dram_tensor("data", [128, ST_W + MV_W + 1024], bf16,
                               kind="ExternalInput").ap()
    out_dram = nc.dram_tensor("out", [128, 2 * U], f32,
                              kind="ExternalOutput").ap()

    with tile.TileContext(nc) as tc, ExitStack() as ctx:
        big = ctx.enter_context(tc.tile_pool(name="big", bufs=1))
        cpool = ctx.enter_context(tc.tile_pool(name="cps", bufs=CPBUFS,
                                               space="PSUM"))
        opool = ctx.enter_context(tc.tile_pool(name="ops", bufs=1,
                                               space="PSUM"))

        st_sb = big.tile([128, ST_W], bf16, tag="st")
        mv_sb = big.tile([128, T * 256], bf16, tag="mv")
        w2_sb = big.tile([128, 1024], bf16, tag="w2")
        c_spread = big.tile([128, B * 512], bf16, tag="csp")
        out_sb = big.tile([128, 2 * U], f32, tag="outsb")

        # per chunk: gpsimd-zero the moving region (slot-separation zeros),
        # then DMA the stationary block + the 4 compact moving bands
        nc.sync.dma_start(w2_sb[:], data_dram[:, ST_W + MV_W:])
        for kc in range(NCH):
            b0, b1 = bounds[kc], bounds[kc + 1]
            nc.gpsimd.memset(mv_sb[:, b0 * 256:b1 * 256], 0.0)
            nc.sync.dma_start(st_sb[:, b0 * 128:b1 * 128],
                              data_dram[:, b0 * 128:b1 * 128])
            for j in range(4):
                dst = mv_sb[32 * j:32 * j + 32, b0 * 256:b1 * 256] \
                    .rearrange("p (t w) -> p t w", w=256)[:, :, 64 * j:64 * j + 64]
                src = data_dram[32 * j:32 * j + 32,
                                ST_W + b0 * 64:ST_W + b1 * 64]
                nc.sync.dma_start(dst, src)

        # C stage: one full-array matmul per 16 positions; one full-width
        # f32->bf16 drain per PSUM bank, alternating DVE/ACT
        ps = None
        for t in range(T):
            if t % 2 == 0:
                ps = cpool.tile([128, 512], f32, tag="cps")
            nc.tensor.matmul(ps[:, 256 * (t % 2):256 * (t % 2) + 256],
                             st_sb[:, t * 128:(t + 1) * 128],
                             mv_sb[:, t * 256:(t + 1) * 256],
                             start=True, stop=True)
            if t % 2 == 1:
                b = t // 2
                eng = nc.vector.tensor_copy if b % 2 == 0 else nc.scalar.copy
                eng(c_spread[:, b * 512:(b + 1) * 512], ps[:])

        # W stage: 4 concurrent strip chains, one per class g:
        #   po_g[m, u] += sum_f W[f, m*16+h] * C[u, g, f, h]   (16 h-phases)
        pos = []
        for g in range(4):
            po = opool.tile([128, U], f32, tag=f"po{g}", name=f"po{g}")
            pos.append(po)
            for h in range(HID):
                nc.tensor.matmul(
                    po[0:MSG, :],
                    w2_sb[32 * g:32 * g + 32, 64 * h:64 * h + 64],
                    c_spread[32 * g:32 * g + 32, (16 * g + h)::64],
                    start=(h == 0), stop=(h == HID - 1),
                    tile_position=(32 * g, 0))
        for g in range(4):
            eng = nc.vector.tensor_copy if g % 2 == 0 else nc.scalar.copy
            eng(out_sb[64 * (g % 2):64 * (g % 2) + MSG,
                       (g // 2) * U:(g // 2 + 1) * U], pos[g][0:MSG, :])
        nc.sync.dma_start(out_dram[:], out_sb[:])
    nc.compile()
    return nc


def _assemble(outs, segs_per_core, NPOS, U):
    out = np.zeros((N_NODES, MSG), dtype=np.float32)
    mrow = np.arange(MSG)[None, :]
    for c in range(NCORES):
        segs = segs_per_core[c]
        P = min(len(segs), NPOS)
        if P == 0:
            continue
        po_sb = outs[c].astype(np.float32)           # [128, 2U]
        p = np.arange(P)
        u, g = p // 4, p % 4
        part = 64 * (g % 2)[:, None] + mrow          # [P, 64]
        col = ((g // 2) * U + u)[:, None]
        pos_rows = po_sb[part, col]                  # [P, 64]
        nodes = np.fromiter((segs[i][0] for i in range(P)), dtype=np.int64,
                            count=P)
        np.add.at(out, nodes, pos_rows)
    return out


def kernel(node_features, edge_features, edge_sources, edge_targets,
           hidden, initial, W, b):
    from concourse.bass_utils import run_bass_kernel_spmd

    edge_targets = np.asarray(edge_targets)
    edge_sources = np.asarray(edge_sources)
    edge_features = np.asarray(edge_features, dtype=np.float32)
    hidden = np.asarray(hidden, dtype=np.float32)
    W = np.asarray(W, dtype=np.float32)
    b = np.asarray(b, dtype=np.float32)

    key = edge_targets.tobytes()
    if key in _CACHE:
        layout, nc = _CACHE[key]
    else:
        layout = _build_layout(edge_targets)
        segs_per_core, NPOS, T, U = layout
        nc = _build_program(T, U)
        _CACHE[key] = (layout, nc)
    segs_per_core, NPOS, T, U = layout

    w2 = _build_w2(W)
    in_maps = []
    for c in range(NCORES):
        data = _pack_core(segs_per_core[c], NPOS, T, w2,
                          edge_features, edge_sources, hidden)
        in_maps.append({"data": data})

    res = run_bass_kernel_spmd(nc, in_maps, list(range(NCORES)))
    outs = [res.results[c]["out"] for c in range(NCORES)]
    out = _assemble(outs, segs_per_core, NPOS, U)

    if np.any(b):
        # bias term: out[n] += (sum_{e->n} hidden[src e]) @ Br,
        # Br[h, m] = b[m*16+h].  (b is all-zero for this problem.)
        Br = b.reshape(MSG, HID).T.astype(np.float32)
        acc = np.zeros((N_NODES, HID), dtype=np.float32)
        np.add.at(acc, edge_targets, hidden[edge_sources])
        out += acc @ Br
    return out


# revision 9
# speedup vs baseline: 2.1121x; 1.2831x over previous
"""Trainium2 Bass kernel for nn_MessageLayer (GNN message passing), 8 NeuronCores.

Reference computation:
    edge_mat = (edge_features @ W + b).reshape(E, 64, 16)
    messages = einsum('emh,eh->em', edge_mat, hidden[edge_sources])
    out      = segment_sum(messages, edge_targets, num_segments=10000)

Algebraic restructure (cuts FLOPs 32x): since aggregation is linear,
    out[n, m] = sum_{f,h} W[f, m*16+h] * C[n, f, h],
    C[n, f, h] = sum_{e: tgt(e)=n} ef[e, f] * hidden[src(e), h]

Structure (v2.1):
Each segment is split into chunks of <=32 edges ("positions"), sorted by
length.  One full-array matmul handles 16 positions at once:
  - K = 128 rows = 4 row-slots of 32 (slot j holds a "quad" = 4 positions)
  - stationary [128, 128]: row r of slot j carries the edge features of the
    4 positions (classes g=0..3) of quad 4t+j at column groups 32g..32g+32
    -- fully dense.
  - moving [128, 256]: slot j's rows carry the 4 source-hidden vectors at
    cols 64j + 16g + h; all other moving entries are ZERO (slot separation),
    provided by a gpsimd memset + per-slot compact DMA.
  - PSUM out [128, 256]: block (32g+f, 64j+16g+h) = C[pos(t,j,g), f, h];
    (g,g') off-diagonal blocks are garbage that simply stays in place.
Drains are single full-width [128, 512] f32->bf16 copies per PSUM bank
(alternating DVE/ACT) into a *spread* c_spread; the W-stage then runs as
4 concurrent 32-row-strip matmul chains (one per class g), each reading
its valid columns directly via the stride-64 pattern
    c_spread[32g:32g+32, (16g+h)::64]   ->  [32, U] per phase h,
against a 4x-replicated W stationary [32f@32g, 64m] (both m-halves at
once, 16 accumulating phases into po_g [64, U] PSUM).

Sharding: node-ownership (scatter-reduce by target): core c owns nodes
[1250c, 1250c+1250) and receives exactly the edges targeting them, so no
collective is needed; host assembles per-position rows into final output.
All tensors bf16 on the wire/SBUF (f32 PSUM accumulate): rel-err ~3.5e-3
vs the 2e-2 gate.
"""
import numpy as np
from contextlib import ExitStack

N_NODES = 10000
N_EDGES = 320000
HID = 16
MSG = 64
EFD = 32
NCORES = 8
NPC = N_NODES // NCORES          # 1250 nodes owned per core
SPLIT = 32                       # max edges per position (row-slot height)
CPBUFS = 4                       # PSUM tiles for C banks (4 + 4 po = 8)

_CACHE = {}


def _bf16():
    import ml_dtypes
    return ml_dtypes.bfloat16


def _build_layout(edge_targets):
    """Per-core position lists (node, edge-ids, len<=32, sorted desc) plus
    the shared SPMD-uniform grid size: T matmuls of 16 positions each."""
    segs_per_core = []
    for c in range(NCORES):
        lo = c * NPC
        mask = (edge_targets >= lo) & (edge_targets < lo + NPC)
        eids = np.nonzero(mask)[0]
        tgt = edge_targets[eids]
        order = np.argsort(tgt, kind="stable")
        eids = eids[order]
        tgt = tgt[order]
        segs = []
        uniq, starts = np.unique(tgt, return_index=True)
        bounds = list(starts) + [len(tgt)]
        for i, n in enumerate(uniq):
            s, e = bounds[i], bounds[i + 1]
            while e - s > SPLIT:
                segs.append((int(n), eids[s:s + SPLIT]))
                s += SPLIT
            segs.append((int(n), eids[s:e]))
        segs.sort(key=lambda t: -len(t[1]))
        segs_per_core.append(segs)

    nseg = max(len(s) for s in segs_per_core)
    NPOS = ((nseg + 31) // 32) * 32      # multiple of 32 -> T even
    T = NPOS // 16                       # one matmul per 16 positions
    U = NPOS // 4                        # quads; po free width
    assert U <= 512, f"U={U} exceeds one PSUM bank"
    return segs_per_core, NPOS, T, U


def _build_w2(W):
    # w2[32g+f, 64h+m] = W[f, m*16+h], replicated across the 4 class groups
    Wr = W.reshape(EFD, MSG, HID).transpose(0, 2, 1)   # [f, h, m]
    blk = np.ascontiguousarray(Wr.reshape(EFD, HID * MSG))
    return np.tile(blk, (4, 1)).astype(np.float32)     # [128, 1024]


def _pack_core(segs, NPOS, T, w2, edge_features, edge_sources, hidden):
    """DRAM image per core: [128, T*128 (stationary) | T*64 (moving compact)
    | 1024 (w2)], bf16.  Position p=(t=p//16, j=(p//4)%4, g=p%4):
      stationary[32j+r, t*128+32g+f] = ef[E_p[r], f]
      moving-compact[32j+r, t*64+16g+h] = hidden[src(E_p[r]), h]
    (moving band j lives at partitions 32j..32j+32 of the same col range)."""
    St = np.zeros((128, T * 128), dtype=np.float32)
    Mv = np.zeros((128, T * 64), dtype=np.float32)
    for p in range(min(len(segs), NPOS)):
        _, eids = segs[p]
        k = len(eids)
        t, j, g = p // 16, (p // 4) % 4, p % 4
        r0 = 32 * j
        St[r0:r0 + k, t * 128 + 32 * g:t * 128 + 32 * g + EFD] = \
            edge_features[eids]
        Mv[r0:r0 + k, t * 64 + 16 * g:t * 64 + 16 * g + HID] = \
            hidden[edge_sources[eids]]
    D = np.concatenate([St, Mv, w2], axis=1)
    return np.ascontiguousarray(D.astype(_bf16()))


def _chunk_bounds(T):
    # small first chunk so the PE starts early
    return [0, max(2, T // 8) & ~1, (T // 3) & ~1, (2 * T // 3) & ~1, T]


def _build_program(T, U):
    import concourse.tile as tile
    from concourse import bacc, mybir

    f32 = mybir.dt.float32
    bf16 = mybir.dt.bfloat16
    ST_W = T * 128
    MV_W = T * 64
    B = T // 2                           # PSUM banks over the C stage
    bounds = _chunk_bounds(T)
    NCH = len(bounds) - 1

    nc = bacc.Bacc("TRN2", target_bir_lowering=False, debug=False,
                   num_devices=NCORES)
    data_dram = nc.dram_tensor("data", [128, ST_W + MV_W + 1024], bf16,
                               kind="ExternalInput").ap()
    out_dram = nc.dram_tensor("out", [128, 2 * U], f32,
                              kind="ExternalOutput").ap()

    with tile.TileContext(nc) as tc, ExitStack() as ctx:
        big = ctx.enter_context(tc.tile_pool(name="big", bufs=1))
        cpool = ctx.enter_context(tc.tile_pool(name="cps", bufs=CPBUFS,
                                               space="PSUM"))
        opool = ctx.enter_context(tc.tile_pool(name="ops", bufs=1,
                                               space="PSUM"))

        st_sb = big.tile([128, ST_W], bf16, tag="st")
        mv_sb = big.tile([128, T * 256], bf16, tag="mv")
        w2_sb = big.tile([128, 1024], bf16, tag="w2")
        c_spread = big.tile([128, B * 512], bf16, tag="csp")
        out_sb = big.tile([128, 2 * U], f32, tag="outsb")

        # zero the moving region (slot-separation zeros) up front, split
        # DVE/gpsimd per chunk so no chunk's DMA waits long; then per chunk
        # DMA the stationary block + the 4 compact moving bands
        for kc in range(NCH):
            b0, b1 = bounds[kc], bounds[kc + 1]
            mid = ((b0 + b1) // 2) & ~1
            nc.vector.memset(mv_sb[:, b0 * 256:mid * 256], 0.0)
            nc.gpsimd.memset(mv_sb[:, mid * 256:b1 * 256], 0.0)
        for kc in range(NCH):
            b0, b1 = bounds[kc], bounds[kc + 1]
            nc.sync.dma_start(st_sb[:, b0 * 128:b1 * 128],
                              data_dram[:, b0 * 128:b1 * 128])
            for j in range(4):
                dst = mv_sb[32 * j:32 * j + 32, b0 * 256:b1 * 256] \
                    .rearrange("p (t w) -> p t w", w=256)[:, :, 64 * j:64 * j + 64]
                src = data_dram[32 * j:32 * j + 32,
                                ST_W + b0 * 64:ST_W + b1 * 64]
                nc.sync.dma_start(dst, src)
            if kc == 0:
                nc.sync.dma_start(w2_sb[:], data_dram[:, ST_W + MV_W:])

        # C stage: one full-array matmul per 16 positions; one full-width
        # f32->bf16 drain per PSUM bank, alternating DVE/ACT
        ps = None
        for t in range(T):
            if t % 2 == 0:
                ps = cpool.tile([128, 512], f32, tag="cps")
            nc.tensor.matmul(ps[:, 256 * (t % 2):256 * (t % 2) + 256],
                             st_sb[:, t * 128:(t + 1) * 128],
                             mv_sb[:, t * 256:(t + 1) * 256],
                             start=True, stop=True)
            if t % 2 == 1:
                b = t // 2
                eng = nc.vector.tensor_copy if b % 2 == 0 else nc.scalar.copy
                eng(c_spread[:, b * 512:(b + 1) * 512], ps[:])

        # W stage: 4 concurrent strip chains, one per class g:
        #   po_g[m, u] += sum_f W[f, m*16+h] * C[u, g, f, h]   (16 h-phases)
        pos = [opool.tile([128, U], f32, tag=f"po{g}", name=f"po{g}")
               for g in range(4)]
        for h in range(HID):
            for g in range(4):       # g inner: 4 strips run concurrently
                nc.tensor.matmul(
                    pos[g][0:MSG, :],
                    w2_sb[32 * g:32 * g + 32, 64 * h:64 * h + 64],
                    c_spread[32 * g:32 * g + 32, (16 * g + h)::64],
                    start=(h == 0), stop=(h == HID - 1),
                    tile_position=(32 * g, 0))
        for g in range(4):
            eng = nc.vector.tensor_copy if g % 2 == 0 else nc.scalar.copy
            eng(out_sb[64 * (g % 2):64 * (g % 2) + MSG,
                       (g // 2) * U:(g // 2 + 1) * U], pos[g][0:MSG, :])
        nc.sync.dma_start(out_dram[:], out_sb[:])
    nc.compile()
    return nc


def _assemble(outs, segs_per_core, NPOS, U):
    out = np.zeros((N_NODES, MSG), dtype=np.float32)
    mrow = np.arange(MSG)[None, :]
    for c in range(NCORES):
        segs = segs_per_core[c]
        P = min(len(segs), NPOS)
        if P == 0:
            continue
        po_sb = outs[c].astype(np.float32)           # [128, 2U]
        p = np.arange(P)
        u, g = p // 4, p % 4
        part = 64 * (g % 2)[:, None] + mrow          # [P, 64]
        col = ((g // 2) * U + u)[:, None]
        pos_rows = po_sb[part, col]                  # [P, 64]
        nodes = np.fromiter((segs[i][0] for i in range(P)), dtype=np.int64,
                            count=P)
        np.add.at(out, nodes, pos_rows)
    return out


def kernel(node_features, edge_features, edge_sources, edge_targets,
           hidden, initial, W, b):
    from concourse.bass_utils import run_bass_kernel_spmd

    edge_targets = np.asarray(edge_targets)
    edge_sources = np.asarray(edge_sources)
    edge_features = np.asarray(edge_features, dtype=np.float32)
    hidden = np.asarray(hidden, dtype=np.float32)
    W = np.asarray(W, dtype=np.float32)
    b = np.asarray(b, dtype=np.float32)

    key = edge_targets.tobytes()
    if key in _CACHE:
        layout, nc = _CACHE[key]
    else:
        layout = _build_layout(edge_targets)
        segs_per_core, NPOS, T, U = layout
        nc = _build_program(T, U)
        _CACHE[key] = (layout, nc)
    segs_per_core, NPOS, T, U = layout

    w2 = _build_w2(W)
    in_maps = []
    for c in range(NCORES):
        data = _pack_core(segs_per_core[c], NPOS, T, w2,
                          edge_features, edge_sources, hidden)
        in_maps.append({"data": data})

    res = run_bass_kernel_spmd(nc, in_maps, list(range(NCORES)))
    outs = [res.results[c]["out"] for c in range(NCORES)]
    out = _assemble(outs, segs_per_core, NPOS, U)

    if np.any(b):
        # bias term: out[n] += (sum_{e->n} hidden[src e]) @ Br,
        # Br[h, m] = b[m*16+h].  (b is all-zero for this problem.)
        Br = b.reshape(MSG, HID).T.astype(np.float32)
        acc = np.zeros((N_NODES, HID), dtype=np.float32)
        np.add.at(acc, edge_targets, hidden[edge_sources])
        out += acc @ Br
    return out


# revision 10
# speedup vs baseline: 2.5561x; 1.2102x over previous
"""Trainium2 Bass kernel for nn_MessageLayer (GNN message passing), 8 NeuronCores.

Reference computation:
    edge_mat = (edge_features @ W + b).reshape(E, 64, 16)
    messages = einsum('emh,eh->em', edge_mat, hidden[edge_sources])
    out      = segment_sum(messages, edge_targets, num_segments=10000)

Algebraic restructure (cuts FLOPs 32x): since aggregation is linear,
    out[n, m] = sum_{f,h} W[f, m*16+h] * C[n, f, h],
    C[n, f, h] = sum_{e: tgt(e)=n} ef[e, f] * hidden[src(e), h]

Structure (v2.3): per-target segments ("positions", split at 64) are packed
into full-array K=128 matmuls in two species:
  - BIG (33..64 edges): 2 row-slots of 64 x 4 ef column-classes
    = 8 positions/matmul, moving [128, 128]
  - SMALL (<=32 edges): 4 row-slots of 32 x 4 classes
    = 16 positions/matmul, moving [128, 256]
Stationary [128, 128]: row r of slot j holds the 4 class-edges' features at
column groups 32g..32g+32 (dense).  Moving: slot j's rows carry the 4
source-hidden vectors at cols 64j+16g+h, zeros elsewhere (slot separation;
zeros memset on-device, data DMA'd compactly per slot-band).
PSUM out: valid C-blocks at (32g+f, stripe 16s+h) with s%4 == g uniformly
across both species, garbage elsewhere.  Each bank (4 big or 2 small
matmuls) drains as two half-width [128, 256] f32->bf16 copies (DVE + ACT in
parallel) into the spread c_spread.
W-stage: 4 concurrent 32-row-strip matmul chains (one per class g), each
reading its valid columns via stride-64:  c_spread[32g:32g+32, (16g+h)::64]
-> [32, U], against a 4x-replicated W stationary [32f@32g, 64m] (both
m-halves at once, 16 accumulating h-phases into po_g [64, U] PSUM).

Sharding: node-ownership (scatter-reduce by target): core c owns nodes
[1250c, 1250c+1250) and receives exactly the edges targeting them, so no
collective is needed; host assembles per-position rows into final output.
All tensors bf16 on the wire/SBUF (f32 PSUM accumulate): rel-err ~3.5e-3
vs the 2e-2 gate.
"""
import numpy as np
from contextlib import ExitStack

N_NODES = 10000
N_EDGES = 320000
HID = 16
MSG = 64
EFD = 32
NCORES = 8
NPC = N_NODES // NCORES          # 1250 nodes owned per core
CPBUFS = 4                       # PSUM tiles for C banks (4 + 4 po = 8)

_CACHE = {}


def _bf16():
    import ml_dtypes
    return ml_dtypes.bfloat16


def _build_layout(edge_targets):
    """Per-core position lists (node, edge-ids, len<=64, sorted desc; all
    len>32 "big" positions precede the "small" ones) plus the SPMD-uniform
    grid: T_big 8-position matmuls then T_small 16-position matmuls."""
    segs_per_core, nbig_per_core = [], []
    for c in range(NCORES):
        lo = c * NPC
        mask = (edge_targets >= lo) & (edge_targets < lo + NPC)
        eids = np.nonzero(mask)[0]
        tgt = edge_targets[eids]
        order = np.argsort(tgt, kind="stable")
        eids = eids[order]
        tgt = tgt[order]
        segs = []
        uniq, starts = np.unique(tgt, return_index=True)
        bounds = list(starts) + [len(tgt)]
        for i, n in enumerate(uniq):
            s, e = bounds[i], bounds[i + 1]
            while e - s > 64:
                segs.append((int(n), eids[s:s + 64]))
                s += 64
            segs.append((int(n), eids[s:e]))
        segs.sort(key=lambda t: -len(t[1]))
        segs_per_core.append(segs)
        nbig_per_core.append(sum(1 for _, e in segs if len(e) > 32))

    T_big = -(-max(nbig_per_core) // 8)
    T_big = ((T_big + 3) // 4) * 4            # whole banks of 4 matmuls
    nsmall = max(len(s) - b for s, b in zip(segs_per_core, nbig_per_core))
    T_small = -(-nsmall // 16)
    T_small = ((T_small + 1) // 2) * 2        # whole banks of 2 matmuls
    U = 2 * T_big + 4 * T_small               # total position quads
    assert U <= 512, f"U={U} exceeds one PSUM bank"
    return segs_per_core, nbig_per_core, T_big, T_small, U


def _build_w2(W):
    # w2[32g+f, 64h+m] = W[f, m*16+h], replicated across the 4 class groups
    Wr = W.reshape(EFD, MSG, HID).transpose(0, 2, 1)   # [f, h, m]
    blk = np.ascontiguousarray(Wr.reshape(EFD, HID * MSG))
    return np.tile(blk, (4, 1)).astype(np.float32)     # [128, 1024]


def _pack_core(segs, nbig, T_big, T_small, w2, edge_features, edge_sources,
               hidden):
    """DRAM image per core, bf16:
      [128, T*128 st | T_big*64 mv-big | T_small*64 mv-small | 1024 w2]
    BIG position p<8*T_big (t=p//8, j=(p//4)%2, g=p%4):
      st[64j+r, t*128+32g+f];  mv-big band j at partitions 64j: [64j+r, t*64+16g+h]
    SMALL position q (t=q//16, j=(q//4)%4, g=q%4):
      st[32j+r, (T_big+t)*128+32g+f];  mv-small band j at partitions 32j."""
    T = T_big + T_small
    St = np.zeros((128, T * 128), dtype=np.float32)
    MvB = np.zeros((128, T_big * 64), dtype=np.float32)
    MvS = np.zeros((128, T_small * 64), dtype=np.float32)
    for i in range(len(segs)):
        _, eids = segs[i]
        k = len(eids)
        if i < nbig:
            t, j, g = i // 8, (i // 4) % 2, i % 4
            r0 = 64 * j
            St[r0:r0 + k, t * 128 + 32 * g:t * 128 + 32 * g + EFD] = \
                edge_features[eids]
            MvB[r0:r0 + k, t * 64 + 16 * g:t * 64 + 16 * g + HID] = \
                hidden[edge_sources[eids]]
        else:
            q = i - nbig
            t, j, g = q // 16, (q // 4) % 4, q % 4
            r0 = 32 * j
            St[r0:r0 + k, (T_big + t) * 128 + 32 * g:
               (T_big + t) * 128 + 32 * g + EFD] = edge_features[eids]
            MvS[r0:r0 + k, t * 64 + 16 * g:t * 64 + 16 * g + HID] = \
                hidden[edge_sources[eids]]
    D = np.concatenate([St, MvB, MvS, w2], axis=1)
    return np.ascontiguousarray(D.astype(_bf16()))


def _chunks(T, n, align):
    bs = [((round(k * T / n)) // align) * align for k in range(n)] + [T]
    bs[1] = max(bs[1], align) if T >= align else bs[1]
    return [(bs[k], bs[k + 1]) for k in range(n) if bs[k + 1] > bs[k]]


def _build_program(T_big, T_small, U):
    import concourse.tile as tile
    from concourse import bacc, mybir

    f32 = mybir.dt.float32
    bf16 = mybir.dt.bfloat16
    T = T_big + T_small
    ST_W = T * 128
    B_big = T_big // 4
    B = B_big + T_small // 2             # total PSUM bank-fills
    MVB_SB = T_big * 128                 # big region width in mv_sb

    nc = bacc.Bacc("TRN2", target_bir_lowering=False, debug=False,
                   num_devices=NCORES)
    data_dram = nc.dram_tensor(
        "data", [128, ST_W + (T_big + T_small) * 64 + 1024], bf16,
        kind="ExternalInput").ap()
    out_dram = nc.dram_tensor("out", [128, 2 * U], f32,
                              kind="ExternalOutput").ap()

    with tile.TileContext(nc) as tc, ExitStack() as ctx:
        big = ctx.enter_context(tc.tile_pool(name="big", bufs=1))
        cpool = ctx.enter_context(tc.tile_pool(name="cps", bufs=CPBUFS,
                                               space="PSUM"))
        opool = ctx.enter_context(tc.tile_pool(name="ops", bufs=1,
                                               space="PSUM"))

        st_sb = big.tile([128, ST_W], bf16, tag="st")
        mv_sb = big.tile([128, T_big * 128 + T_small * 256], bf16, tag="mv")
        w2_sb = big.tile([128, 1024], bf16, tag="w2")
        c_spread = big.tile([128, B * 512], bf16, tag="csp")
        out_sb = big.tile([128, 2 * U], f32, tag="outsb")

        bchunks = _chunks(T_big, 3, 4)
        schunks = _chunks(T_small, 2, 2)

        # slot-separation zeros: big region on DVE, small region on gpsimd
        # (both finish well before the drains need those engines)
        for b0, b1 in bchunks:
            nc.vector.memset(mv_sb[:, b0 * 128:b1 * 128], 0.0)
        for b0, b1 in schunks:
            nc.gpsimd.memset(mv_sb[:, MVB_SB + b0 * 256:MVB_SB + b1 * 256],
                             0.0)

        # stationary + w2 DMAs on the ACT HWDGE queue, moving bands on SP
        for b0, b1 in bchunks:
            nc.scalar.dma_start(st_sb[:, b0 * 128:b1 * 128],
                                data_dram[:, b0 * 128:b1 * 128])
            for j in range(2):
                dst = mv_sb[64 * j:64 * j + 64, b0 * 128:b1 * 128] \
                    .rearrange("p (t w) -> p t w", w=128)[:, :, 64 * j:64 * j + 64]
                src = data_dram[64 * j:64 * j + 64,
                                ST_W + b0 * 64:ST_W + b1 * 64]
                nc.sync.dma_start(dst, src)
        nc.scalar.dma_start(w2_sb[:], data_dram[:, ST_W + T * 64:])
        for b0, b1 in schunks:
            nc.scalar.dma_start(
                st_sb[:, (T_big + b0) * 128:(T_big + b1) * 128],
                data_dram[:, (T_big + b0) * 128:(T_big + b1) * 128])
            for j in range(4):
                dst = mv_sb[32 * j:32 * j + 32,
                            MVB_SB + b0 * 256:MVB_SB + b1 * 256] \
                    .rearrange("p (t w) -> p t w", w=256)[:, :, 64 * j:64 * j + 64]
                src = data_dram[32 * j:32 * j + 32,
                                ST_W + T_big * 64 + b0 * 64:
                                ST_W + T_big * 64 + b1 * 64]
                nc.sync.dma_start(dst, src)

        # C stage; each bank drains as two parallel half-copies (DVE + ACT)
        def drain(ps, b):
            nc.vector.tensor_copy(c_spread[:, b * 512:b * 512 + 256],
                                  ps[:, 0:256])
            nc.scalar.copy(c_spread[:, b * 512 + 256:b * 512 + 512],
                           ps[:, 256:512])

        ps = None
        for t in range(T_big):
            if t % 4 == 0:
                ps = cpool.tile([128, 512], f32, tag="cps")
            nc.tensor.matmul(ps[:, 128 * (t % 4):128 * (t % 4) + 128],
                             st_sb[:, t * 128:(t + 1) * 128],
                             mv_sb[:, t * 128:(t + 1) * 128],
                             start=True, stop=True)
            if t % 4 == 3:
                drain(ps, t // 4)
        for ts in range(T_small):
            if ts % 2 == 0:
                ps = cpool.tile([128, 512], f32, tag="cps")
            nc.tensor.matmul(
                ps[:, 256 * (ts % 2):256 * (ts % 2) + 256],
                st_sb[:, (T_big + ts) * 128:(T_big + ts + 1) * 128],
                mv_sb[:, MVB_SB + ts * 256:MVB_SB + (ts + 1) * 256],
                start=True, stop=True)
            if ts % 2 == 1:
                drain(ps, B_big + ts // 2)

        # W stage: po_g[m, u] += sum_f W[f, m*16+h] * C[u, g, f, h]
        pos = [opool.tile([128, U], f32, tag=f"po{g}", name=f"po{g}")
               for g in range(4)]
        for h in range(HID):
            for g in range(4):       # g inner: 4 strips run concurrently
                nc.tensor.matmul(
                    pos[g][0:MSG, :],
                    w2_sb[32 * g:32 * g + 32, 64 * h:64 * h + 64],
                    c_spread[32 * g:32 * g + 32, (16 * g + h)::64],
                    start=(h == 0), stop=(h == HID - 1),
                    tile_position=(32 * g, 0))
        for g in range(4):
            eng = nc.vector.tensor_copy if g % 2 == 0 else nc.scalar.copy
            eng(out_sb[64 * (g % 2):64 * (g % 2) + MSG,
                       (g // 2) * U:(g // 2 + 1) * U], pos[g][0:MSG, :])
        nc.sync.dma_start(out_dram[:], out_sb[:])
    nc.compile()
    return nc


def _assemble(outs, segs_per_core, nbig_per_core, T_big, U):
    out = np.zeros((N_NODES, MSG), dtype=np.float32)
    mrow = np.arange(MSG)[None, :]
    for c in range(NCORES):
        segs = segs_per_core[c]
        nbig = nbig_per_core[c]
        P = len(segs)
        if P == 0:
            continue
        po_sb = outs[c].astype(np.float32)           # [128, 2U]
        i = np.arange(P)
        p = np.where(i < nbig, i, 8 * T_big + (i - nbig))  # grid position
        u, g = p // 4, p % 4
        part = 64 * (g % 2)[:, None] + mrow          # [P, 64]
        col = ((g // 2) * U + u)[:, None]
        pos_rows = po_sb[part, col]                  # [P, 64]
        nodes = np.fromiter((segs[k][0] for k in range(P)), dtype=np.int64,
                            count=P)
        np.add.at(out, nodes, pos_rows)
    return out


def kernel(node_features, edge_features, edge_sources, edge_targets,
           hidden, initial, W, b):
    from concourse.bass_utils import run_bass_kernel_spmd

    edge_targets = np.asarray(edge_targets)
    edge_sources = np.asarray(edge_sources)
    edge_features = np.asarray(edge_features, dtype=np.float32)
    hidden = np.asarray(hidden, dtype=np.float32)
    W = np.asarray(W, dtype=np.float32)
    b = np.asarray(b, dtype=np.float32)

    key = edge_targets.tobytes()
    if key in _CACHE:
        layout, nc = _CACHE[key]
    else:
        layout = _build_layout(edge_targets)
        segs_per_core, nbig_per_core, T_big, T_small, U = layout
        nc = _build_program(T_big, T_small, U)
        _CACHE[key] = (layout, nc)
    segs_per_core, nbig_per_core, T_big, T_small, U = layout

    w2 = _build_w2(W)
    in_maps = []
    for c in range(NCORES):
        data = _pack_core(segs_per_core[c], nbig_per_core[c], T_big, T_small,
                          w2, edge_features, edge_sources, hidden)
        in_maps.append({"data": data})

    res = run_bass_kernel_spmd(nc, in_maps, list(range(NCORES)))
    outs = [res.results[c]["out"] for c in range(NCORES)]
    out = _assemble(outs, segs_per_core, nbig_per_core, T_big, U)

    if np.any(b):
        # bias term: out[n] += (sum_{e->n} hidden[src e]) @ Br,
        # Br[h, m] = b[m*16+h].  (b is all-zero for this problem.)
        Br = b.reshape(MSG, HID).T.astype(np.float32)
        acc = np.zeros((N_NODES, HID), dtype=np.float32)
        np.add.at(acc, edge_targets, hidden[edge_sources])
        out += acc @ Br
    return out


# revision 12
# speedup vs baseline: 2.7917x; 1.0922x over previous
"""Trainium2 Bass kernel for nn_MessageLayer (GNN message passing), 8 NeuronCores.

Reference computation:
    edge_mat = (edge_features @ W + b).reshape(E, 64, 16)
    messages = einsum('emh,eh->em', edge_mat, hidden[edge_sources])
    out      = segment_sum(messages, edge_targets, num_segments=10000)

Algebraic restructure (cuts FLOPs 32x): since aggregation is linear,
    out[n, m] = sum_{f,h} W[f, m*16+h] * C[n, f, h],
    C[n, f, h] = sum_{e: tgt(e)=n} ef[e, f] * hidden[src(e), h]

Structure (v2.3): per-target segments ("positions", split at 64) are packed
into full-array K=128 matmuls in two species:
  - BIG (33..64 edges): 2 row-slots of 64 x 4 ef column-classes
    = 8 positions/matmul, moving [128, 128]
  - SMALL (<=32 edges): 4 row-slots of 32 x 4 classes
    = 16 positions/matmul, moving [128, 256]
Stationary [128, 128]: row r of slot j holds the 4 class-edges' features at
column groups 32g..32g+32 (dense).  Moving: slot j's rows carry the 4
source-hidden vectors at cols 64j+16g+h, zeros elsewhere (slot separation;
zeros memset on-device, data DMA'd compactly per slot-band).
PSUM out: valid C-blocks at (32g+f, stripe 16s+h) with s%4 == g uniformly
across both species, garbage elsewhere.  Each bank (4 big or 2 small
matmuls) drains as two half-width [128, 256] f32->bf16 copies (DVE + ACT in
parallel) into the spread c_spread.
W-stage: 4 concurrent 32-row-strip matmul chains (one per class g), each
reading its valid columns via stride-64:  c_spread[32g:32g+32, (16g+h)::64]
-> [32, U], against a 4x-replicated W stationary [32f@32g, 64m] (both
m-halves at once, 16 accumulating h-phases into po_g [64, U] PSUM).

Sharding: node-ownership (scatter-reduce by target): core c owns nodes
[1250c, 1250c+1250) and receives exactly the edges targeting them, so no
collective is needed; host assembles per-position rows into final output.
All tensors bf16 on the wire/SBUF (f32 PSUM accumulate): rel-err ~3.5e-3
vs the 2e-2 gate.
"""
import numpy as np
from contextlib import ExitStack

N_NODES = 10000
N_EDGES = 320000
HID = 16
MSG = 64
EFD = 32
NCORES = 8
NPC = N_NODES // NCORES          # 1250 nodes owned per core
CPBUFS = 4                       # PSUM tiles for C banks (4 + 4 po = 8)

_CACHE = {}


def _bf16():
    import ml_dtypes
    return ml_dtypes.bfloat16


def _build_layout(edge_targets):
    """Per-core position lists (node, edge-ids, len<=64, sorted desc; all
    len>32 "big" positions precede the "small" ones) plus the SPMD-uniform
    grid: T_big 8-position matmuls then T_small 16-position matmuls."""
    segs_per_core, nbig_per_core = [], []
    for c in range(NCORES):
        lo = c * NPC
        mask = (edge_targets >= lo) & (edge_targets < lo + NPC)
        eids = np.nonzero(mask)[0]
        tgt = edge_targets[eids]
        order = np.argsort(tgt, kind="stable")
        eids = eids[order]
        tgt = tgt[order]
        segs = []
        uniq, starts = np.unique(tgt, return_index=True)
        bounds = list(starts) + [len(tgt)]
        for i, n in enumerate(uniq):
            s, e = bounds[i], bounds[i + 1]
            while e - s > 64:
                segs.append((int(n), eids[s:s + 64]))
                s += 64
            segs.append((int(n), eids[s:e]))
        segs.sort(key=lambda t: -len(t[1]))
        segs_per_core.append(segs)
        nbig_per_core.append(sum(1 for _, e in segs if len(e) > 32))

    T_big = -(-max(nbig_per_core) // 8)
    T_big = ((T_big + 3) // 4) * 4            # whole banks of 4 matmuls
    nsmall = max(len(s) - b for s, b in zip(segs_per_core, nbig_per_core))
    T_small = -(-nsmall // 16)
    T_small = ((T_small + 1) // 2) * 2        # whole banks of 2 matmuls
    U = 2 * T_big + 4 * T_small               # total position quads
    assert U <= 512, f"U={U} exceeds one PSUM bank"
    return segs_per_core, nbig_per_core, T_big, T_small, U


def _build_w2(W):
    # w2[32g+f, 64h+m] = W[f, m*16+h], replicated across the 4 class groups
    Wr = W.reshape(EFD, MSG, HID).transpose(0, 2, 1)   # [f, h, m]
    blk = np.ascontiguousarray(Wr.reshape(EFD, HID * MSG))
    return np.tile(blk, (4, 1)).astype(np.float32)     # [128, 1024]


def _pack_core(segs, nbig, T_big, T_small, w2, edge_features, edge_sources,
               hidden):
    """DRAM image per core, bf16:
      [128, T*128 st | T_big*64 mv-big | T_small*64 mv-small | 1024 w2]
    BIG position p<8*T_big (t=p//8, j=(p//4)%2, g=p%4):
      st[64j+r, t*128+32g+f];  mv-big band j at partitions 64j: [64j+r, t*64+16g+h]
    SMALL position q (t=q//16, j=(q//4)%4, g=q%4):
      st[32j+r, (T_big+t)*128+32g+f];  mv-small band j at partitions 32j."""
    T = T_big + T_small
    St = np.zeros((128, T * 128), dtype=np.float32)
    MvB = np.zeros((128, T_big * 64), dtype=np.float32)
    MvS = np.zeros((128, T_small * 64), dtype=np.float32)
    for i in range(len(segs)):
        _, eids = segs[i]
        k = len(eids)
        if i < nbig:
            t, j, g = i // 8, (i // 4) % 2, i % 4
            r0 = 64 * j
            St[r0:r0 + k, t * 128 + 32 * g:t * 128 + 32 * g + EFD] = \
                edge_features[eids]
            MvB[r0:r0 + k, t * 64 + 16 * g:t * 64 + 16 * g + HID] = \
                hidden[edge_sources[eids]]
        else:
            q = i - nbig
            t, j, g = q // 16, (q // 4) % 4, q % 4
            r0 = 32 * j
            St[r0:r0 + k, (T_big + t) * 128 + 32 * g:
               (T_big + t) * 128 + 32 * g + EFD] = edge_features[eids]
            MvS[r0:r0 + k, t * 64 + 16 * g:t * 64 + 16 * g + HID] = \
                hidden[edge_sources[eids]]
    D = np.concatenate([St, MvB, MvS, w2], axis=1)
    return np.ascontiguousarray(D.astype(_bf16()))


def _chunks(T, n, align):
    bs = [((round(k * T / n)) // align) * align for k in range(n)] + [T]
    bs[1] = max(bs[1], align) if T >= align else bs[1]
    return [(bs[k], bs[k + 1]) for k in range(n) if bs[k + 1] > bs[k]]


def _build_program(T_big, T_small, U):
    import concourse.tile as tile
    from concourse import bacc, mybir

    f32 = mybir.dt.float32
    bf16 = mybir.dt.bfloat16
    T = T_big + T_small
    ST_W = T * 128
    B_big = T_big // 4
    B = B_big + T_small // 2             # total PSUM bank-fills
    MVB_SB = T_big * 128                 # big region width in mv_sb

    nc = bacc.Bacc("TRN2", target_bir_lowering=False, debug=False,
                   num_devices=NCORES)
    data_dram = nc.dram_tensor(
        "data", [128, ST_W + (T_big + T_small) * 64 + 1024], bf16,
        kind="ExternalInput").ap()
    out_dram = nc.dram_tensor("out", [128, 2 * U], f32,
                              kind="ExternalOutput").ap()

    with tile.TileContext(nc) as tc, ExitStack() as ctx:
        big = ctx.enter_context(tc.tile_pool(name="big", bufs=1))
        cpool = ctx.enter_context(tc.tile_pool(name="cps", bufs=CPBUFS,
                                               space="PSUM"))
        opool = ctx.enter_context(tc.tile_pool(name="ops", bufs=1,
                                               space="PSUM"))

        st_sb = big.tile([128, ST_W], bf16, tag="st")
        # moving data is BAND-MAJOR: per-band contiguous regions (cheap DMA);
        # the matmul rhs reads across bands with a strided AP instead.
        mv_sb = big.tile([128, (2 * T_big + 4 * T_small) * 64], bf16,
                         tag="mv")
        w2_sb = big.tile([128, 1024], bf16, tag="w2")
        c_spread = big.tile([128, B * 512], bf16, tag="csp")
        out_sb = big.tile([128, 2 * U], f32, tag="outsb")
        wu_sb = big.tile([1, 8], bf16, tag="wu")

        # PE warm-up: keep the tensor engine busy through the DMA head so
        # HAM un-throttles (1.2 -> 2.4 GHz) before the real matmuls start
        nc.vector.memset(wu_sb[:], 1.0)
        wups = opool.tile([128, U], f32, tag="po0", name="po0_wu")
        for _ in range(60):
            nc.tensor.matmul(wups[0:1, 0:1], wu_sb[0:1, 0:1], wu_sb[0:1, 1:2],
                             start=True, stop=True)

        bchunks = _chunks(T_big, 3, 4)
        schunks = _chunks(T_small, 2, 2)

        # slot-separation zeros (full band regions; the band DMA then
        # overwrites its own rows), split DVE / gpsimd
        for b0, b1 in bchunks:
            nc.vector.memset(mv_sb[:, b0 * 64:b1 * 64], 0.0)
            nc.gpsimd.memset(
                mv_sb[:, MVB_SB // 2 + b0 * 64:MVB_SB // 2 + b1 * 64], 0.0)
        for b0, b1 in schunks:
            for j in range(4):
                off = MVB_SB + (j * T_small + b0) * 64
                eng = nc.vector if j % 2 else nc.gpsimd
                eng.memset(mv_sb[:, off:off + (b1 - b0) * 64], 0.0)

        # stationary + w2 DMAs on the ACT HWDGE queue, moving bands on SP;
        # all band DMAs are contiguous rectangles
        for b0, b1 in bchunks:
            nc.scalar.dma_start(st_sb[:, b0 * 128:b1 * 128],
                                data_dram[:, b0 * 128:b1 * 128])
            for j in range(2):
                off = j * (MVB_SB // 2)
                nc.sync.dma_start(
                    mv_sb[64 * j:64 * j + 64, off + b0 * 64:off + b1 * 64],
                    data_dram[64 * j:64 * j + 64,
                              ST_W + b0 * 64:ST_W + b1 * 64])
        nc.scalar.dma_start(w2_sb[:], data_dram[:, ST_W + T * 64:])
        for b0, b1 in schunks:
            nc.scalar.dma_start(
                st_sb[:, (T_big + b0) * 128:(T_big + b1) * 128],
                data_dram[:, (T_big + b0) * 128:(T_big + b1) * 128])
            for j in range(4):
                off = MVB_SB + (j * T_small + b0) * 64
                nc.sync.dma_start(
                    mv_sb[32 * j:32 * j + 32, off:off + (b1 - b0) * 64],
                    data_dram[32 * j:32 * j + 32,
                              ST_W + T_big * 64 + b0 * 64:
                              ST_W + T_big * 64 + b1 * 64])

        # C stage; each bank drains as two parallel half-copies (DVE + ACT)
        def drain(ps, b):
            nc.vector.tensor_copy(c_spread[:, b * 512:b * 512 + 256],
                                  ps[:, 0:256])
            nc.scalar.copy(c_spread[:, b * 512 + 256:b * 512 + 512],
                           ps[:, 256:512])

        mv_big = mv_sb[:, 0:MVB_SB].rearrange("p (r c) -> p r c", r=2)
        mv_small = mv_sb[:, MVB_SB:].rearrange("p (r c) -> p r c", r=4)
        ps = None
        for t in range(T_big):
            if t % 4 == 0:
                ps = cpool.tile([128, 512], f32, tag="cps")
            nc.tensor.matmul(ps[:, 128 * (t % 4):128 * (t % 4) + 128],
                             st_sb[:, t * 128:(t + 1) * 128],
                             mv_big[:, :, t * 64:(t + 1) * 64],
                             start=True, stop=True)
            if t % 4 == 3:
                drain(ps, t // 4)
        for ts in range(T_small):
            if ts % 2 == 0:
                ps = cpool.tile([128, 512], f32, tag="cps")
            nc.tensor.matmul(
                ps[:, 256 * (ts % 2):256 * (ts % 2) + 256],
                st_sb[:, (T_big + ts) * 128:(T_big + ts + 1) * 128],
                mv_small[:, :, ts * 64:(ts + 1) * 64],
                start=True, stop=True)
            if ts % 2 == 1:
                drain(ps, B_big + ts // 2)

        # W stage: po_g[m, u] += sum_f W[f, m*16+h] * C[u, g, f, h]
        pos = [opool.tile([128, U], f32, tag=f"po{g}", name=f"po{g}")
               for g in range(4)]
        for h in range(HID):
            for g in range(4):       # g inner: 4 strips run concurrently
                nc.tensor.matmul(
                    pos[g][0:MSG, :],
                    w2_sb[32 * g:32 * g + 32, 64 * h:64 * h + 64],
                    c_spread[32 * g:32 * g + 32, (16 * g + h)::64],
                    start=(h == 0), stop=(h == HID - 1),
                    tile_position=(32 * g, 0))
        for g in range(4):
            eng = nc.vector.tensor_copy if g % 2 == 0 else nc.scalar.copy
            eng(out_sb[64 * (g % 2):64 * (g % 2) + MSG,
                       (g // 2) * U:(g // 2 + 1) * U], pos[g][0:MSG, :])
        nc.sync.dma_start(out_dram[:], out_sb[:])
    nc.compile()
    return nc


def _assemble(outs, segs_per_core, nbig_per_core, T_big, U):
    out = np.zeros((N_NODES, MSG), dtype=np.float32)
    mrow = np.arange(MSG)[None, :]
    for c in range(NCORES):
        segs = segs_per_core[c]
        nbig = nbig_per_core[c]
        P = len(segs)
        if P == 0:
            continue
        po_sb = outs[c].astype(np.float32)           # [128, 2U]
        i = np.arange(P)
        p = np.where(i < nbig, i, 8 * T_big + (i - nbig))  # grid position
        u, g = p // 4, p % 4
        part = 64 * (g % 2)[:, None] + mrow          # [P, 64]
        col = ((g // 2) * U + u)[:, None]
        pos_rows = po_sb[part, col]                  # [P, 64]
        nodes = np.fromiter((segs[k][0] for k in range(P)), dtype=np.int64,
                            count=P)
        np.add.at(out, nodes, pos_rows)
    return out


def kernel(node_features, edge_features, edge_sources, edge_targets,
           hidden, initial, W, b):
    from concourse.bass_utils import run_bass_kernel_spmd

    edge_targets = np.asarray(edge_targets)
    edge_sources = np.asarray(edge_sources)
    edge_features = np.asarray(edge_features, dtype=np.float32)
    hidden = np.asarray(hidden, dtype=np.float32)
    W = np.asarray(W, dtype=np.float32)
    b = np.asarray(b, dtype=np.float32)

    key = edge_targets.tobytes()
    if key in _CACHE:
        layout, nc = _CACHE[key]
    else:
        layout = _build_layout(edge_targets)
        segs_per_core, nbig_per_core, T_big, T_small, U = layout
        nc = _build_program(T_big, T_small, U)
        _CACHE[key] = (layout, nc)
    segs_per_core, nbig_per_core, T_big, T_small, U = layout

    w2 = _build_w2(W)
    in_maps = []
    for c in range(NCORES):
        data = _pack_core(segs_per_core[c], nbig_per_core[c], T_big, T_small,
                          w2, edge_features, edge_sources, hidden)
        in_maps.append({"data": data})

    res = run_bass_kernel_spmd(nc, in_maps, list(range(NCORES)))
    outs = [res.results[c]["out"] for c in range(NCORES)]
    out = _assemble(outs, segs_per_core, nbig_per_core, T_big, U)

    if np.any(b):
        # bias term: out[n] += (sum_{e->n} hidden[src e]) @ Br,
        # Br[h, m] = b[m*16+h].  (b is all-zero for this problem.)
        Br = b.reshape(MSG, HID).T.astype(np.float32)
        acc = np.zeros((N_NODES, HID), dtype=np.float32)
        np.add.at(acc, edge_targets, hidden[edge_sources])
        out += acc @ Br
    return out
